# revision 6
# baseline (speedup 1.0000x reference)
"""Trainium2 Bass kernel for nn_Attention_2216203124924 (sparse/varlen GQA attention).

Full computation:
  xq/xk/xv = x @ {wq,wk,wv}.T ; per-head RMSNorm(q,k) ; RoPE via
  rope_cache[positions] ; GQA repeat ; per-segment causal attention
  (segments from cu_seqlens) ; out @ wo.T

Distribution (8 NeuronCores, tensor-parallel over heads):
  core c owns q-heads [4c,4c+4) and kv-head c (GQA groups align),
  wo is row-sharded; each core emits a partial [2048,4096] bf16 output and
  the host sums the 8 partials in f32.

On-device layout is "transposed" ([feature, seq]) throughout so the
contraction dim always sits on SBUF partitions. The kernel is a software
pipeline over 512-column m-chunks, structured to keep the PE dense (no
>3.4us idle windows, so the HAM clock gate stays at 8/8):

  Phase 1: per chunk, weight-stationary qkv projection (PSUM accumulators
  over 32 contraction tiles) runs dense on the PE while the PREVIOUS
  chunk's RMS stats + RoPE (DVE chains, host-gathered cos/sin, PE
  swap-half permutation) and V transposes execute on the other engines.

  Phase 2: per chunk, scoresT = kT.T @ q per key tile, unnormalized exp
  (scores are O(1)), compile-time segment mask plan, PV accumulated in
  PSUM; the softmax denominator is a gpsimd partition_all_reduce (no PSUM
  banks, no PE); normalization on DVE. The previous chunk's output
  projection is issued after each chunk's attention so its dense matmuls
  fill the attention chain gaps. Chunk 0's attention overlaps the last
  rope block via early release of the projection PSUM pool.

All matmul operands are bf16 or f32r (full PE rate). The segment/causal
structure from cu_seqlens and the rope gather by positions are resolved on
the host at build time; the NEFF is specialized to them.
"""

import os
import sys

import numpy as np

for _p in ("/opt/trn_rl_repo",):
    if os.path.isdir(_p) and _p not in sys.path:
        sys.path.insert(0, _p)

S = 2048
D = 4096
HD = 128
HALF = 64
N_HEADS = 32
N_KV = 8
NCORES = 8
QH = N_HEADS // NCORES          # 4 q heads per core
NO = QH + 2                     # o-tiles per core in qkv projection: q0..q3, k, v
DT = D // 128                   # 32 contraction tiles
MC = S // 512                   # 4 m-chunks of 512
NT = S // 128                   # 16 key tiles
EPS = 1e-6
SCALE = HD ** -0.5

LAST_RESULT = None  # BassKernelResults of the most recent run (for test harness)


def _attention_plan(cu_seqlens):
    """Compile-time mask plan from cu_seqlens.

    Returns (plan, mask_pack):
      plan[mc] = list of (nt, w0, w1, mask_ops); w0/w1 are column offsets
      (multiples of 128, relative to the 512-wide m-chunk) of the contiguous
      valid window; mask_ops = [(j, kind, idx)] for 128-col subtiles needing
      a multiplicative 0/1 mask: kind 'tri' uses the shared causal triangle,
      kind 'host' uses mask_pack[:, idx*128:(idx+1)*128].
    """
    idx = np.arange(S)
    seg = np.searchsorted(np.asarray(cu_seqlens), idx, side="right") - 1
    mask_qk = (seg[:, None] == seg[None, :]) & (idx[:, None] >= idx[None, :])
    mask_t = mask_qk.T  # [n, m]

    plan = []
    tiles = []
    tile_ids = {}
    for mc in range(MC):
        entries = []
        for nt in range(NT):
            blk = mask_t[nt * 128:(nt + 1) * 128, mc * 512:(mc + 1) * 512]
            if not blk.any():
                continue
            js = [j for j in range(4) if blk[:, j * 128:(j + 1) * 128].any()]
            jlo, jhi = min(js), max(js)
            assert js == list(range(jlo, jhi + 1)), "valid window not contiguous"
            mops = []
            for j in range(jlo, jhi + 1):
                sub = blk[:, j * 128:(j + 1) * 128]
                if sub.all():
                    continue
                m0g = mc * 512 + j * 128
                n0g = nt * 128
                if m0g == n0g and np.array_equal(
                    sub, idx[:128][None, :] >= idx[:128][:, None]
                ):
                    mops.append((j, "tri", -1))
                else:
                    key = sub.tobytes()
                    if key not in tile_ids:
                        tile_ids[key] = len(tiles)
                        tiles.append(sub.astype(np.float32))
                    mops.append((j, "host", tile_ids[key]))
            entries.append((nt, jlo * 128, (jhi + 1) * 128, mops))
        assert entries, "every query row attends to at least itself"
        plan.append(entries)

    if tiles:
        mask_pack = np.concatenate(tiles, axis=1)
    else:
        mask_pack = np.zeros((128, 128), dtype=np.float32)
    return plan, np.ascontiguousarray(mask_pack)


def _build_graph(plan, n_mask_cols):
    import concourse.bass as bass  # noqa: PLC0415
    import concourse.mybir as mybir  # noqa: PLC0415
    import concourse.tile as tile  # noqa: PLC0415
    from concourse import bacc, bass_isa  # noqa: PLC0415
    from contextlib import ExitStack  # noqa: PLC0415

    f32 = mybir.dt.float32
    f32r = mybir.dt.float32r
    bf16 = mybir.dt.bfloat16
    AF = mybir.ActivationFunctionType

    nc = bacc.Bacc()
    xT_p = nc.declare_dram_parameter("xT", [D, S], bf16, isOutput=False)
    wqkv_p = nc.declare_dram_parameter("w_qkv", [128, NO * DT * 128], bf16, isOutput=False)
    wo_p = nc.declare_dram_parameter("w_o", [128, QH * D], bf16, isOutput=False)
    cs_p = nc.declare_dram_parameter("cs", [128, 4 * S], bf16, isOutput=False)
    consts_p = nc.declare_dram_parameter("consts", [128, 2 * 128], f32r, isOutput=False)
    constsb_p = nc.declare_dram_parameter(
        "consts_bf", [128, 3 * 128 + n_mask_cols], bf16, isOutput=False
    )
    out_p = nc.declare_dram_parameter("out", [S, D], bf16, isOutput=True)

    with tile.TileContext(nc) as tc, ExitStack() as ctx:
        const = ctx.enter_context(tc.tile_pool(name="const", bufs=1))
        persist = ctx.enter_context(tc.tile_pool(name="persist", bufs=1))
        # rope/stats small-matmul bank; outlives phase 1 (the last rope block
        # overlaps phase 2's first attention)
        psm = ctx.enter_context(tc.tile_pool(name="smpsum", bufs=1, space="PSUM"))

        consts = const.tile([128, 2 * 128], f32r)
        ones_col = consts[:, 0:1]
        ones_row = consts[0:1, 0:128]
        sca_row = consts[0:1, 128:256]  # all = HD**0.5 (divide-by folds attn scale)

        constsb = const.tile([128, 3 * 128 + n_mask_cols], bf16)
        swp_bf = constsb[:, 128:256]    # swap-halves permutation
        ident_bf = constsb[:, 256:384]  # identity (for PE transpose)
        mask_bf = constsb[:, 384:]

        eps_col = const.tile([128, 1], f32)
        nc.gpsimd.memset(eps_col[:], EPS)

        # full-S persistents: rope'd Q heads and K, transposed V (all bf16)
        kT = persist.tile([128, S], bf16)
        v_sb = persist.tile([128, S], bf16)
        wo_sb = persist.tile([128, QH * D], bf16)
        qbf = [persist.tile([128, S], bf16, tag=f"qbf{h}", name=f"qbf{h}") for h in range(QH)]

        # rotating per-(head, m-chunk) attention-output tiles
        pat = ctx.enter_context(tc.tile_pool(name="attn", bufs=2))

        nc.scalar.dma_start(consts[:], consts_p[:])
        nc.scalar.dma_start(constsb[:], constsb_p[:])

        with ExitStack() as s1:
            # w chunks split across gpsimd (even) / scalar (odd), in d order,
            # so the projection d-loop is never starved.
            pw = s1.enter_context(tc.tile_pool(name="wqkv", bufs=1))
            w_sb = pw.tile([128, NO * DT * 128], bf16)
            NCH = 16
            wchunk = NO * DT * 128 // NCH
            for wci in range(NCH):
                eng = nc.gpsimd if wci % 2 == 0 else nc.scalar
                eng.dma_start(
                    w_sb[:, wci * wchunk:(wci + 1) * wchunk],
                    wqkv_p[:, wci * wchunk:(wci + 1) * wchunk],
                )

            px = s1.enter_context(tc.tile_pool(name="xstream", bufs=6))
            pcs = s1.enter_context(tc.tile_pool(name="csstream", bufs=2))
            pqk = s1.enter_context(tc.tile_pool(name="qkvrot", bufs=2))
            prs = s1.enter_context(tc.tile_pool(name="rsq", bufs=3))
            pscr = s1.enter_context(tc.tile_pool(name="scratch", bufs=2))

            qkv_hist = {}
            cst_hist = {}

            def rope_block(mc):
                """RMS stats + rope + V transposes for chunk mc (inputs were
                produced by proj(mc); runs under proj(mc+1)'s matmul cover)."""
                msl = slice(mc * 512, (mc + 1) * 512)
                qkv = qkv_hist.pop(mc)
                cst = cst_hist.pop(mc)
                for o in range(QH + 1):
                    sq = pscr.tile([128, 512], f32r, tag="sq", name="sq")
                    nc.vector.tensor_mul(sq[:], qkv[o][:], qkv[o][:])
                    ss = psm.tile([1, 512], f32, tag="sm", name="ss", padded_shape=[128, 512])
                    nc.tensor.matmul(ss[:], ones_col, sq[:], start=True, stop=True)
                    rsq = prs.tile([1, 512], f32r, tag="rsq", name="rsq")
                    nc.scalar.activation(
                        rsq[:], ss[:], AF.Sqrt, bias=eps_col[0:1, :], scale=1.0 / HD,
                    )

                    csb = 0 if o < QH else 2
                    row = sca_row if o < QH else ones_row
                    bp = psm.tile([128, 512], f32, tag="sm", name="bp")
                    nc.tensor.matmul(bp[:], swp_bf, qkv[o][:], start=True, stop=True)
                    t1 = pscr.tile([128, 512], f32, tag="t1", name="t1")
                    nc.vector.tensor_mul(t1[:], qkv[o][:], cst[csb][:])
                    t2 = pscr.tile([128, 512], f32, tag="t2", name="t2")
                    nc.vector.tensor_mul(t2[:], bp[:], cst[csb + 1][:])
                    nc.vector.tensor_add(t1[:], t1[:], t2[:])
                    bc = psm.tile([128, 512], f32, tag="sm", name="bc")
                    nc.tensor.matmul(bc[:], row, rsq[:], start=True, stop=True)
                    rrb = pscr.tile([128, 512], f32, tag="rrb", name="rrb")
                    nc.vector.reciprocal_approx_fast(out=rrb[:], in_=bc[:])
                    dst = qbf[o][:, msl] if o < QH else kT[:, msl]
                    nc.vector.tensor_mul(dst, t1[:], rrb[:])

                for k in range(4):
                    nt = mc * 4 + k
                    tp = psm.tile([128, 128], bf16, tag="sm", name="tp")
                    nc.tensor.transpose(tp[:], qkv[QH + 1][:, k * 128:(k + 1) * 128], ident_bf)
                    nc.vector.tensor_copy(v_sb[:, nt * 128:(nt + 1) * 128], tp[:])

            # ---- phase 1: projection d-loops, rope pipelined one chunk behind
            with ExitStack() as s1p:
                pq = s1p.enter_context(tc.tile_pool(name="qkvpsum", bufs=1, space="PSUM"))
                for mc in range(MC):
                    msl = slice(mc * 512, (mc + 1) * 512)
                    cst = {}
                    for ci in range(4):
                        t = pcs.tile([128, 512], bf16, tag=f"cs{ci}", name=f"cs{ci}")
                        nc.scalar.dma_start(
                            t[:], cs_p[:, ci * S + mc * 512: ci * S + (mc + 1) * 512]
                        )
                        cst[ci] = t
                    cst_hist[mc] = cst

                    accs = [
                        pq.tile([128, 512], f32, tag=f"acc{o}", name=f"acc{o}",
                                bufs=2 if o == 0 else 1)
                        for o in range(NO)
                    ]
                    for d in range(DT):
                        xt = px.tile([128, 512], bf16, tag="xt")
                        nc.sync.dma_start(xt[:], xT_p[d * 128:(d + 1) * 128, msl])
                        for o in range(NO):
                            woff = (d * NO + o) * 128
                            nc.tensor.matmul(
                                accs[o][:],
                                w_sb[:, woff:woff + 128],
                                xt[:],
                                start=(d == 0),
                                stop=(d == DT - 1),
                            )

                    qkv = []
                    for o in range(NO):
                        t = pqk.tile([128, 512], bf16, tag=f"qk{o}", name=f"qk{o}")
                        nc.vector.tensor_copy(t[:], accs[o][:])
                        qkv.append(t)
                    qkv_hist[mc] = qkv

                    if mc >= 1:
                        rope_block(mc - 1)
                    if mc == 2:
                        # wo prefetch after the startup DMA crunch is over
                        woch = QH * D // 4
                        for wci in range(4):
                            nc.scalar.dma_start(
                                wo_sb[:, wci * woch:(wci + 1) * woch],
                                wo_p[:, wci * woch:(wci + 1) * woch],
                            )
            # pq released here: phase-2 PSUM pools take its banks, so chunk
            # 0's attention can overlap the final rope block below.

            # ---------------- phase 2: attention + output projection ----------------
            with ExitStack() as s2:
                psco = s2.enter_context(tc.tile_pool(name="scpsum", bufs=3, space="PSUM"))
                pov = s2.enter_context(tc.tile_pool(name="ovpsum", bufs=1, space="PSUM"))
                pyp = s2.enter_context(tc.tile_pool(name="ypsum", bufs=3, space="PSUM"))
                pex = s2.enter_context(tc.tile_pool(name="exsbuf", bufs=3))
                pnr = s2.enter_context(tc.tile_pool(name="nrsbuf", bufs=2))
                pys = s2.enter_context(tc.tile_pool(name="ysbuf", bufs=2))

                def attn_block(mc):
                    entries = plan[mc]
                    n_ent = len(entries)
                    attnT = []
                    for h in range(QH):
                        ov = pov.tile([128, 512], f32, tag="ov")
                        dsum = pnr.tile([128, 512], f32, tag="dsum")
                        cover0 = None
                        for i, (nt, w0, w1, mops) in enumerate(entries):
                            nsl = slice(nt * 128, (nt + 1) * 128)
                            sc = psco.tile([128, 512], f32, tag="sc")
                            nc.tensor.matmul(
                                sc[:, w0:w1], kT[:, nsl],
                                qbf[h][:, mc * 512 + w0: mc * 512 + w1],
                                start=True, stop=True,
                            )
                            ex = pex.tile([128, 512], bf16, tag="ex")
                            nc.scalar.activation(ex[:, w0:w1], sc[:, w0:w1], AF.Exp)
                            for (j, kind, tix) in mops:
                                jsl = slice(j * 128, (j + 1) * 128)
                                if kind == "tri":
                                    # zero strictly-below-diagonal (m < n) entries
                                    nc.gpsimd.affine_select(
                                        out=ex[:, jsl], in_=ex[:, jsl],
                                        compare_op=mybir.AluOpType.is_ge,
                                        fill=0.0, base=0,
                                        pattern=[[1, 128]], channel_multiplier=-1,
                                    )
                                else:
                                    nc.vector.tensor_mul(
                                        ex[:, jsl], ex[:, jsl],
                                        mask_bf[:, tix * 128:(tix + 1) * 128],
                                    )
                            nc.tensor.matmul(
                                ov[:, w0:w1], v_sb[:, nsl], ex[:, w0:w1],
                                start=(i == 0), stop=(i == n_ent - 1),
                                skip_group_check=True,
                            )
                            # denominator: partition-sum of exp on gpsimd
                            if i == 0:
                                nc.gpsimd.partition_all_reduce(
                                    dsum[:, w0:w1], ex[:, w0:w1], 128,
                                    bass_isa.ReduceOp.add,
                                )
                                cover0 = (w0, w1)
                                if w0 > 0:
                                    nc.gpsimd.memset(dsum[:, 0:w0], 0.0)
                                if w1 < 512:
                                    nc.gpsimd.memset(dsum[:, w1:512], 0.0)
                            else:
                                dtmp = pnr.tile([128, 512], f32, tag="dtmp")
                                nc.gpsimd.partition_all_reduce(
                                    dtmp[:, w0:w1], ex[:, w0:w1], 128,
                                    bass_isa.ReduceOp.add,
                                )
                                nc.vector.tensor_add(
                                    dsum[:, w0:w1], dsum[:, w0:w1], dtmp[:, w0:w1]
                                )
                        rrb2 = pnr.tile([128, 512], f32, tag="rrb2")
                        nc.vector.reciprocal_approx_fast(out=rrb2[:], in_=dsum[:])
                        at = pat.tile([128, 512], bf16, tag=f"attnT{h}", name=f"attnT{h}")
                        nc.vector.tensor_mul(at[:], ov[:], rrb2[:])
                        attnT.append(at)
                    return attnT

                def outproj(mc, attnT):
                    for j in range(4):
                        mt = mc * 4 + j
                        tsl = slice(mt * 128, (mt + 1) * 128)
                        jsl = slice(j * 128, (j + 1) * 128)
                        ys = pys.tile([128, D], bf16, tag="ys", name="ys")
                        for ec in range(D // 512):
                            yp = pyp.tile([128, 512], f32, tag="yp", name="yp")
                            for t in range(QH):
                                nc.tensor.matmul(
                                    yp[:],
                                    attnT[t][:, jsl],
                                    wo_sb[:, t * D + ec * 512: t * D + (ec + 1) * 512],
                                    start=(t == 0),
                                    stop=(t == QH - 1),
                                )
                            esl = slice(ec * 512, (ec + 1) * 512)
                            if ec % 2 == 0:
                                nc.scalar.activation(ys[:, esl], yp[:], AF.Copy)
                            else:
                                nc.vector.tensor_copy(ys[:, esl], yp[:])
                        nc.sync.dma_start(out_p[tsl, :], ys[:])

                attnT_hist = {0: attn_block(0)}
                rope_block(MC - 1)
                for mc in range(1, MC):
                    attnT_hist[mc] = attn_block(mc)
                    outproj(mc - 1, attnT_hist.pop(mc - 1))
                outproj(MC - 1, attnT_hist.pop(MC - 1))

    nc.finalize()
    return nc


def kernel(x, wq, wk, wv, wo, q_norm_w, k_norm_w, rope_cache, positions, cu_seqlens):
    global LAST_RESULT
    from concourse.bass_utils import run_bass_kernel_spmd  # noqa: PLC0415

    x = np.asarray(x, dtype=np.float32)
    wq = np.asarray(wq, dtype=np.float32)
    wk = np.asarray(wk, dtype=np.float32)
    wv = np.asarray(wv, dtype=np.float32)
    wo = np.asarray(wo, dtype=np.float32)
    q_norm_w = np.asarray(q_norm_w, dtype=np.float32)
    k_norm_w = np.asarray(k_norm_w, dtype=np.float32)
    rope_cache = np.asarray(rope_cache, dtype=np.float32)
    positions = np.asarray(positions)
    cu_seqlens = np.asarray(cu_seqlens)

    import ml_dtypes  # noqa: PLC0415

    # ---- host prep (shared) ----
    xT = np.ascontiguousarray(x[0].T.astype(ml_dtypes.bfloat16))  # [D, S]

    pos = positions.reshape(-1)
    cs = rope_cache[pos]               # [S, HALF, 2]
    cosT = cs[:, :, 0].T               # [HALF, S]
    sinT = cs[:, :, 1].T
    cs1 = np.concatenate([cosT, cosT], axis=0)    # [128, S]
    cs2 = np.concatenate([-sinT, sinT], axis=0)

    def fold(w):
        w = w.reshape(HD, 1)
        wsw = np.concatenate([w[HALF:], w[:HALF]], axis=0)
        return cs1 * w, cs2 * wsw

    cs1q, cs2q = fold(q_norm_w)
    cs1k, cs2k = fold(k_norm_w)
    cs_host = np.ascontiguousarray(
        np.concatenate([cs1q, cs2q, cs1k, cs2k], axis=1).astype(ml_dtypes.bfloat16)
    )  # [128, 4S]

    plan, mask_pack = _attention_plan(cu_seqlens)

    consts_bf = np.zeros((128, 3 * 128 + mask_pack.shape[1]), dtype=np.float32)
    consts_bf[:, 0:128] = 1.0
    swp = np.zeros((128, 128), dtype=np.float32)
    swp[np.arange(128), (np.arange(128) + HALF) % 128] = 1.0
    consts_bf[:, 128:256] = swp
    consts_bf[:, 256:384] = np.eye(128, dtype=np.float32)
    consts_bf[:, 384:] = mask_pack
    consts_bf = consts_bf.astype(ml_dtypes.bfloat16)

    consts = np.zeros((128, 2 * 128), dtype=np.float32)
    consts[:, 0:128] = 1.0
    consts[:, 128:256] = 1.0 / SCALE

    # ---- per-core weight shards ----
    in_maps = []
    for c in range(NCORES):
        w_all = np.concatenate(
            [
                wq[c * QH * HD:(c + 1) * QH * HD],   # [512, D]
                wk[c * HD:(c + 1) * HD],             # [128, D]
                wv[c * HD:(c + 1) * HD],             # [128, D]
            ],
            axis=0,
        )  # [NO*128, D]
        w_host = np.ascontiguousarray(
            w_all.reshape(NO, 128, DT, 128).transpose(3, 2, 0, 1)
            .reshape(128, NO * DT * 128).astype(ml_dtypes.bfloat16)
        )
        wo_c = wo[:, c * QH * HD:(c + 1) * QH * HD].T  # [512, D]
        wo_host = np.ascontiguousarray(
            wo_c.reshape(QH, 128, D).transpose(1, 0, 2)
            .reshape(128, QH * D).astype(ml_dtypes.bfloat16)
        )
        in_maps.append(
            {
                "xT": xT,
                "w_qkv": w_host,
                "w_o": wo_host,
                "cs": cs_host,
                "consts": consts,
                "consts_bf": consts_bf,
            }
        )

    nc = _build_graph(plan, mask_pack.shape[1])
    res = run_bass_kernel_spmd(nc, in_maps, list(range(NCORES)))
    LAST_RESULT = res

    out = res.results[0]["out"].astype(np.float32)
    for c in range(1, NCORES):
        out = out + res.results[c]["out"].astype(np.float32)
    return out.reshape(1, S, D)


# revision 12
# speedup vs baseline: 1.3264x; 1.3264x over previous
"""Trainium2 Bass kernel for nn_Attention_2216203124924 (sparse/varlen GQA attention).

Full computation:
  xq/xk/xv = x @ {wq,wk,wv}.T ; per-head RMSNorm(q,k) ; RoPE via
  rope_cache[positions] ; GQA repeat ; per-segment causal attention
  (segments from cu_seqlens) ; out @ wo.T

Distribution (8 NeuronCores, tensor-parallel over heads):
  core c owns q-heads [4c,4c+4) and kv-head c (GQA groups align),
  wo is row-sharded; each core emits a partial [2048,4096] bf16 output and
  the host sums the 8 partials in f32.

On-device layout is "transposed" ([feature, seq]) throughout so the
contraction dim always sits on SBUF partitions. The kernel is a software
pipeline over 512-column m-chunks, structured to keep the PE dense (no
>3.4us idle windows, so the HAM clock gate stays at 8/8):

  Phase 1: per chunk, weight-stationary qkv projection (PSUM accumulators
  over 32 contraction tiles) runs dense on the PE while the PREVIOUS
  chunk's RMS stats + RoPE (DVE chains, host-gathered cos/sin, PE
  swap-half permutation) and V transposes execute on the other engines.

  Phase 2: per chunk, scoresT = kT.T @ q per key tile, unnormalized exp
  (scores are O(1)), compile-time segment mask plan, PV accumulated in
  PSUM; the softmax denominator is a gpsimd partition_all_reduce (no PSUM
  banks, no PE); normalization on DVE. The previous chunk's output
  projection is issued after each chunk's attention so its dense matmuls
  fill the attention chain gaps. Chunk 0's attention overlaps the last
  rope block via early release of the projection PSUM pool.

All matmul operands are bf16 or f32r (full PE rate). The segment/causal
structure from cu_seqlens and the rope gather by positions are resolved on
the host at build time; the NEFF is specialized to them.
"""

import os
import sys

import numpy as np

for _p in ("/opt/trn_rl_repo",):
    if os.path.isdir(_p) and _p not in sys.path:
        sys.path.insert(0, _p)

S = 2048
D = 4096
HD = 128
HALF = 64
N_HEADS = 32
N_KV = 8
NCORES = 8
QH = N_HEADS // NCORES          # 4 q heads per core
NO = QH + 2                     # o-tiles per core in qkv projection: q0..q3, k, v
DT = D // 128                   # 32 contraction tiles
MC = S // 512                   # 4 m-chunks of 512
NT = S // 128                   # 16 key tiles
EPS = 1e-6
SCALE = HD ** -0.5

LAST_RESULT = None  # BassKernelResults of the most recent run (for test harness)


def _attention_plan(cu_seqlens):
    """Compile-time mask plan from cu_seqlens.

    Returns (plan, mask_pack):
      plan[mc] = list of (nt, w0, w1, mask_ops); w0/w1 are column offsets
      (multiples of 128, relative to the 512-wide m-chunk) of the contiguous
      valid window; mask_ops = [(j, kind, idx)] for 128-col subtiles needing
      a multiplicative 0/1 mask: kind 'tri' uses the shared causal triangle,
      kind 'host' uses mask_pack[:, idx*128:(idx+1)*128].
    """
    idx = np.arange(S)
    seg = np.searchsorted(np.asarray(cu_seqlens), idx, side="right") - 1
    mask_qk = (seg[:, None] == seg[None, :]) & (idx[:, None] >= idx[None, :])
    mask_t = mask_qk.T  # [n, m]

    plan = []
    tiles = []
    tile_ids = {}
    for mc in range(MC):
        entries = []
        for nt in range(NT):
            blk = mask_t[nt * 128:(nt + 1) * 128, mc * 512:(mc + 1) * 512]
            if not blk.any():
                continue
            js = [j for j in range(4) if blk[:, j * 128:(j + 1) * 128].any()]
            jlo, jhi = min(js), max(js)
            assert js == list(range(jlo, jhi + 1)), "valid window not contiguous"
            mops = []
            for j in range(jlo, jhi + 1):
                sub = blk[:, j * 128:(j + 1) * 128]
                if sub.all():
                    continue
                m0g = mc * 512 + j * 128
                n0g = nt * 128
                if m0g == n0g and np.array_equal(
                    sub, idx[:128][None, :] >= idx[:128][:, None]
                ):
                    mops.append((j, "tri", -1))
                else:
                    key = sub.tobytes()
                    if key not in tile_ids:
                        tile_ids[key] = len(tiles)
                        tiles.append(sub.astype(np.float32))
                    mops.append((j, "host", tile_ids[key]))
            entries.append((nt, jlo * 128, (jhi + 1) * 128, mops))
        assert entries, "every query row attends to at least itself"
        plan.append(entries)

    if tiles:
        mask_pack = np.concatenate(tiles, axis=1)
    else:
        mask_pack = np.zeros((128, 128), dtype=np.float32)
    return plan, np.ascontiguousarray(mask_pack)


def _build_graph(plan, n_mask_cols):
    import concourse.bass as bass  # noqa: PLC0415
    import concourse.mybir as mybir  # noqa: PLC0415
    import concourse.tile as tile  # noqa: PLC0415
    from concourse import bacc, bass_isa  # noqa: PLC0415
    from contextlib import ExitStack  # noqa: PLC0415

    f32 = mybir.dt.float32
    f32r = mybir.dt.float32r
    bf16 = mybir.dt.bfloat16
    AF = mybir.ActivationFunctionType

    nc = bacc.Bacc()
    xT_p = nc.declare_dram_parameter("xT", [D, S], bf16, isOutput=False)
    wqkv_p = nc.declare_dram_parameter("w_qkv", [128, NO * DT * 128], bf16, isOutput=False)
    wo_p = nc.declare_dram_parameter("w_o", [128, QH * D], bf16, isOutput=False)
    cs_p = nc.declare_dram_parameter("cs", [128, 4 * S], bf16, isOutput=False)
    consts_p = nc.declare_dram_parameter("consts", [128, 2 * 128], f32r, isOutput=False)
    constsb_p = nc.declare_dram_parameter(
        "consts_bf", [128, 3 * 128 + n_mask_cols], bf16, isOutput=False
    )
    out_p = nc.declare_dram_parameter("out", [S, D], bf16, isOutput=True)

    with tile.TileContext(nc) as tc, ExitStack() as ctx:
        const = ctx.enter_context(tc.tile_pool(name="const", bufs=1))
        persist = ctx.enter_context(tc.tile_pool(name="persist", bufs=1))
        # rope/stats small-matmul bank; outlives phase 1 (the last rope block
        # overlaps phase 2's first attention)
        psm = ctx.enter_context(tc.tile_pool(name="smpsum", bufs=1, space="PSUM"))

        consts = const.tile([128, 2 * 128], f32r)
        ones_col = consts[:, 0:1]
        ones_row = consts[0:1, 0:128]
        sca_row = consts[0:1, 128:256]  # all = HD**0.5 (divide-by folds attn scale)

        constsb = const.tile([128, 3 * 128 + n_mask_cols], bf16)
        ones_col_bf = constsb[:, 0:1]
        swp_bf = constsb[:, 128:256]    # swap-halves permutation
        ident_bf = constsb[:, 256:384]  # identity (for PE transpose)
        mask_bf = constsb[:, 384:]

        eps_col = const.tile([128, 1], f32)
        nc.gpsimd.memset(eps_col[:], EPS)

        # full-S persistents: rope'd Q heads and K, transposed V (all bf16)
        kT = persist.tile([128, S], bf16)
        v_sb = persist.tile([128, S], bf16)
        wo_sb = persist.tile([128, QH * D], bf16)
        qbf = [persist.tile([128, S], bf16, tag=f"qbf{h}", name=f"qbf{h}") for h in range(QH)]

        # rotating per-(head, m-chunk) attention-output tiles
        pat = ctx.enter_context(tc.tile_pool(name="attn", bufs=2))

        nc.scalar.dma_start(consts[:], consts_p[:])
        nc.scalar.dma_start(constsb[:], constsb_p[:])

        with ExitStack() as s1:
            # w chunks split across gpsimd (even) / scalar (odd), in d order,
            # so the projection d-loop is never starved.
            pw = s1.enter_context(tc.tile_pool(name="wqkv", bufs=1))
            w_sb = pw.tile([128, NO * DT * 128], bf16)
            NCH = 32
            wchunk = NO * DT * 128 // NCH
            for wci in range(NCH):
                eng = nc.gpsimd if wci % 2 == 0 else nc.scalar
                eng.dma_start(
                    w_sb[:, wci * wchunk:(wci + 1) * wchunk],
                    wqkv_p[:, wci * wchunk:(wci + 1) * wchunk],
                )

            px = s1.enter_context(tc.tile_pool(name="xstream", bufs=8))
            pcs = s1.enter_context(tc.tile_pool(name="csstream", bufs=2))
            pqk = s1.enter_context(tc.tile_pool(name="qkvrot", bufs=2))
            prs = s1.enter_context(tc.tile_pool(name="rsq", bufs=3))
            pscr = s1.enter_context(tc.tile_pool(name="scratch", bufs=2))

            qkv_hist = {}
            cst_hist = {}
            rsq_hist = {}

            def stats_block(mc):
                """RMS stats (sum-of-squares matmul + scalar Sqrt) for chunk
                mc. All Sqrt activations are issued in phase 1 so the scalar
                activation table never thrashes against phase 2's Exp."""
                qkv = qkv_hist[mc]
                rsqs = []
                for o in range(QH + 1):
                    sq = pscr.tile([128, 512], f32r, tag="sq", name="sq")
                    nc.vector.tensor_mul(sq[:], qkv[o][:], qkv[o][:])
                    ss = psm.tile([1, 512], f32, tag="sm", name="ss", padded_shape=[128, 512])
                    nc.tensor.matmul(ss[:], ones_col, sq[:], start=True, stop=True)
                    rsq = prs.tile([1, 512], f32r, tag="rsq", name="rsq", bufs=6)
                    nc.scalar.activation(
                        rsq[:], ss[:], AF.Sqrt, bias=eps_col[0:1, :], scale=1.0 / HD,
                    )
                    rsqs.append(rsq)
                rsq_hist[mc] = rsqs

            def rot_block(mc):
                """RoPE rotation + V transposes for chunk mc (no scalar ops)."""
                msl = slice(mc * 512, (mc + 1) * 512)
                qkv = qkv_hist.pop(mc)
                cst = cst_hist.pop(mc)
                rsqs = rsq_hist.pop(mc)
                for o in range(QH + 1):
                    csb = 0 if o < QH else 2
                    row = sca_row if o < QH else ones_row
                    bp = psm.tile([128, 512], f32, tag="sm", name="bp")
                    nc.tensor.matmul(bp[:], swp_bf, qkv[o][:], start=True, stop=True)
                    t1 = pscr.tile([128, 512], f32, tag="t1", name="t1")
                    nc.vector.tensor_mul(t1[:], qkv[o][:], cst[csb][:])
                    t2 = pscr.tile([128, 512], f32, tag="t2", name="t2")
                    nc.vector.tensor_mul(t2[:], bp[:], cst[csb + 1][:])
                    nc.vector.tensor_add(t1[:], t1[:], t2[:])
                    bc = psm.tile([128, 512], f32, tag="sm", name="bc")
                    nc.tensor.matmul(bc[:], row, rsqs[o][:], start=True, stop=True)
                    rrb = pscr.tile([128, 512], f32, tag="rrb", name="rrb")
                    nc.vector.reciprocal_approx_fast(out=rrb[:], in_=bc[:])
                    dst = qbf[o][:, msl] if o < QH else kT[:, msl]
                    nc.vector.tensor_mul(dst, t1[:], rrb[:])

                for k in range(4):
                    nt = mc * 4 + k
                    tp = psm.tile([128, 128], bf16, tag="sm", name="tp")
                    nc.tensor.transpose(tp[:], qkv[QH + 1][:, k * 128:(k + 1) * 128], ident_bf)
                    nc.vector.tensor_copy(v_sb[:, nt * 128:(nt + 1) * 128], tp[:])

            # ---- phase 1: projection d-loops, rope pipelined one chunk behind
            with ExitStack() as s1p:
                pq = s1p.enter_context(tc.tile_pool(name="qkvpsum", bufs=1, space="PSUM"))
                for mc in range(MC):
                    msl = slice(mc * 512, (mc + 1) * 512)
                    cst = {}
                    for ci in range(4):
                        t = pcs.tile([128, 512], bf16, tag=f"cs{ci}", name=f"cs{ci}")
                        nc.scalar.dma_start(
                            t[:], cs_p[:, ci * S + mc * 512: ci * S + (mc + 1) * 512]
                        )
                        cst[ci] = t
                    cst_hist[mc] = cst

                    accs = [
                        pq.tile([128, 512], f32, tag=f"acc{o}", name=f"acc{o}",
                                bufs=2 if o == 0 else 1)
                        for o in range(NO)
                    ]
                    for d in range(DT):
                        xt = px.tile([128, 512], bf16, tag="xt")
                        nc.sync.dma_start(xt[:], xT_p[d * 128:(d + 1) * 128, msl])
                        for o in range(NO):
                            woff = (d * NO + o) * 128
                            nc.tensor.matmul(
                                accs[o][:],
                                w_sb[:, woff:woff + 128],
                                xt[:],
                                start=(d == 0),
                                stop=(d == DT - 1),
                            )

                    qkv = []
                    for o in range(NO):
                        t = pqk.tile([128, 512], bf16, tag=f"qk{o}", name=f"qk{o}")
                        nc.vector.tensor_copy(t[:], accs[o][:])
                        qkv.append(t)
                    qkv_hist[mc] = qkv

                    if mc >= 1:
                        stats_block(mc - 1)
                        rot_block(mc - 1)
                    if mc == 2:
                        # wo prefetch after the startup DMA crunch is over
                        woch = QH * D // 4
                        for wci in range(4):
                            nc.scalar.dma_start(
                                wo_sb[:, wci * woch:(wci + 1) * woch],
                                wo_p[:, wci * woch:(wci + 1) * woch],
                            )
                # last chunk's stats now: every scalar Sqrt precedes phase
                # 2's first Exp, so the activation table loads only twice.
                stats_block(MC - 1)
            # pq released here: phase-2 PSUM pools take its banks, so chunk
            # 0's attention can overlap the final rope block below.

            # ---------------- phase 2: attention + output projection ----------------
            with ExitStack() as s2:
                psco = s2.enter_context(tc.tile_pool(name="scpsum", bufs=2, space="PSUM"))
                pov = s2.enter_context(tc.tile_pool(name="ovpsum", bufs=1, space="PSUM"))
                pden = s2.enter_context(tc.tile_pool(name="denpsum", bufs=1, space="PSUM"))
                pbc = s2.enter_context(tc.tile_pool(name="bcpsum", bufs=1, space="PSUM"))
                pyp = s2.enter_context(tc.tile_pool(name="ypsum", bufs=2, space="PSUM"))
                pex = s2.enter_context(tc.tile_pool(name="exsbuf", bufs=3))
                pnr = s2.enter_context(tc.tile_pool(name="nrsbuf", bufs=2))
                pys = s2.enter_context(tc.tile_pool(name="ysbuf", bufs=2))

                def attn_block(mc):
                    entries = plan[mc]
                    n_ent = len(entries)
                    attnT = []
                    for h in range(QH):
                        ov = pov.tile([128, 512], f32, tag="ov")
                        den = pden.tile([1, 512], f32, tag="den", padded_shape=[128, 512])
                        for i, (nt, w0, w1, mops) in enumerate(entries):
                            nsl = slice(nt * 128, (nt + 1) * 128)
                            sc = psco.tile([128, 512], f32, tag="sc")
                            nc.tensor.matmul(
                                sc[:, w0:w1], kT[:, nsl],
                                qbf[h][:, mc * 512 + w0: mc * 512 + w1],
                                start=True, stop=True,
                            )
                            ex = pex.tile([128, 512], bf16, tag="ex")
                            nc.scalar.activation(ex[:, w0:w1], sc[:, w0:w1], AF.Exp)
                            for (j, kind, tix) in mops:
                                jsl = slice(j * 128, (j + 1) * 128)
                                if kind == "tri":
                                    # zero strictly-below-diagonal (m < n) entries
                                    nc.gpsimd.affine_select(
                                        out=ex[:, jsl], in_=ex[:, jsl],
                                        compare_op=mybir.AluOpType.is_ge,
                                        fill=0.0, base=0,
                                        pattern=[[1, 128]], channel_multiplier=-1,
                                    )
                                else:
                                    nc.vector.tensor_mul(
                                        ex[:, jsl], ex[:, jsl],
                                        mask_bf[:, tix * 128:(tix + 1) * 128],
                                    )
                            first = i == 0
                            last = i == n_ent - 1
                            nc.tensor.matmul(
                                ov[:, w0:w1], v_sb[:, nsl], ex[:, w0:w1],
                                start=first, stop=last, skip_group_check=True,
                            )
                            nc.tensor.matmul(
                                den[0:1, w0:w1], ones_col_bf, ex[:, w0:w1],
                                start=first, stop=last, skip_group_check=True,
                            )
                        den_sb = pnr.tile([1, 512], f32r, tag="den_sb")
                        nc.vector.tensor_copy(den_sb[:], den[:])
                        bc2 = pbc.tile([128, 512], f32, tag="bc2")
                        nc.tensor.matmul(bc2[:], ones_row, den_sb[:], start=True, stop=True)
                        rrb2 = pnr.tile([128, 512], f32, tag="rrb2")
                        nc.vector.reciprocal_approx_fast(out=rrb2[:], in_=bc2[:])
                        at = pat.tile([128, 512], bf16, tag=f"attnT{h}", name=f"attnT{h}")
                        nc.vector.tensor_mul(at[:], ov[:], rrb2[:])
                        attnT.append(at)
                    return attnT

                def outproj(mc, attnT):
                    for j in range(4):
                        mt = mc * 4 + j
                        tsl = slice(mt * 128, (mt + 1) * 128)
                        jsl = slice(j * 128, (j + 1) * 128)
                        ys = pys.tile([128, D], bf16, tag="ys", name="ys")
                        for ec in range(D // 512):
                            yp = pyp.tile([128, 512], f32, tag="yp", name="yp")
                            for t in range(QH):
                                nc.tensor.matmul(
                                    yp[:],
                                    attnT[t][:, jsl],
                                    wo_sb[:, t * D + ec * 512: t * D + (ec + 1) * 512],
                                    start=(t == 0),
                                    stop=(t == QH - 1),
                                )
                            esl = slice(ec * 512, (ec + 1) * 512)
                            # DVE-only while Exp activations are still in
                            # flight (a scalar Copy would thrash the ACT
                            # table); the tail chunk can use both engines.
                            if mc == MC - 1 and ec % 2 == 0:
                                nc.scalar.activation(ys[:, esl], yp[:], AF.Copy)
                            else:
                                nc.vector.tensor_copy(ys[:, esl], yp[:])
                        nc.sync.dma_start(out_p[tsl, :], ys[:])

                attnT_hist = {0: attn_block(0)}
                rot_block(MC - 1)
                for mc in range(1, MC):
                    attnT_hist[mc] = attn_block(mc)
                    outproj(mc - 1, attnT_hist.pop(mc - 1))
                outproj(MC - 1, attnT_hist.pop(MC - 1))

    nc.finalize()
    return nc


def kernel(x, wq, wk, wv, wo, q_norm_w, k_norm_w, rope_cache, positions, cu_seqlens):
    global LAST_RESULT
    from concourse.bass_utils import run_bass_kernel_spmd  # noqa: PLC0415

    x = np.asarray(x, dtype=np.float32)
    wq = np.asarray(wq, dtype=np.float32)
    wk = np.asarray(wk, dtype=np.float32)
    wv = np.asarray(wv, dtype=np.float32)
    wo = np.asarray(wo, dtype=np.float32)
    q_norm_w = np.asarray(q_norm_w, dtype=np.float32)
    k_norm_w = np.asarray(k_norm_w, dtype=np.float32)
    rope_cache = np.asarray(rope_cache, dtype=np.float32)
    positions = np.asarray(positions)
    cu_seqlens = np.asarray(cu_seqlens)

    import ml_dtypes  # noqa: PLC0415

    # ---- host prep (shared) ----
    xT = np.ascontiguousarray(x[0].T.astype(ml_dtypes.bfloat16))  # [D, S]

    pos = positions.reshape(-1)
    cs = rope_cache[pos]               # [S, HALF, 2]
    cosT = cs[:, :, 0].T               # [HALF, S]
    sinT = cs[:, :, 1].T
    cs1 = np.concatenate([cosT, cosT], axis=0)    # [128, S]
    cs2 = np.concatenate([-sinT, sinT], axis=0)

    def fold(w):
        w = w.reshape(HD, 1)
        wsw = np.concatenate([w[HALF:], w[:HALF]], axis=0)
        return cs1 * w, cs2 * wsw

    cs1q, cs2q = fold(q_norm_w)
    cs1k, cs2k = fold(k_norm_w)
    cs_host = np.ascontiguousarray(
        np.concatenate([cs1q, cs2q, cs1k, cs2k], axis=1).astype(ml_dtypes.bfloat16)
    )  # [128, 4S]

    plan, mask_pack = _attention_plan(cu_seqlens)

    consts_bf = np.zeros((128, 3 * 128 + mask_pack.shape[1]), dtype=np.float32)
    consts_bf[:, 0:128] = 1.0
    swp = np.zeros((128, 128), dtype=np.float32)
    swp[np.arange(128), (np.arange(128) + HALF) % 128] = 1.0
    consts_bf[:, 128:256] = swp
    consts_bf[:, 256:384] = np.eye(128, dtype=np.float32)
    consts_bf[:, 384:] = mask_pack
    consts_bf = consts_bf.astype(ml_dtypes.bfloat16)

    consts = np.zeros((128, 2 * 128), dtype=np.float32)
    consts[:, 0:128] = 1.0
    consts[:, 128:256] = 1.0 / SCALE

    # ---- per-core weight shards ----
    in_maps = []
    for c in range(NCORES):
        w_all = np.concatenate(
            [
                wq[c * QH * HD:(c + 1) * QH * HD],   # [512, D]
                wk[c * HD:(c + 1) * HD],             # [128, D]
                wv[c * HD:(c + 1) * HD],             # [128, D]
            ],
            axis=0,
        )  # [NO*128, D]
        w_host = np.ascontiguousarray(
            w_all.reshape(NO, 128, DT, 128).transpose(3, 2, 0, 1)
            .reshape(128, NO * DT * 128).astype(ml_dtypes.bfloat16)
        )
        wo_c = wo[:, c * QH * HD:(c + 1) * QH * HD].T  # [512, D]
        wo_host = np.ascontiguousarray(
            wo_c.reshape(QH, 128, D).transpose(1, 0, 2)
            .reshape(128, QH * D).astype(ml_dtypes.bfloat16)
        )
        in_maps.append(
            {
                "xT": xT,
                "w_qkv": w_host,
                "w_o": wo_host,
                "cs": cs_host,
                "consts": consts,
                "consts_bf": consts_bf,
            }
        )

    nc = _build_graph(plan, mask_pack.shape[1])
    res = run_bass_kernel_spmd(nc, in_maps, list(range(NCORES)))
    LAST_RESULT = res

    out = res.results[0]["out"].astype(np.float32)
    for c in range(1, NCORES):
        out = out + res.results[c]["out"].astype(np.float32)
    return out.reshape(1, S, D)


# revision 17
# speedup vs baseline: 1.3829x; 1.0426x over previous
"""Trainium2 Bass kernel for nn_Attention_2216203124924 (sparse/varlen GQA attention).

Full computation:
  xq/xk/xv = x @ {wq,wk,wv}.T ; per-head RMSNorm(q,k) ; RoPE via
  rope_cache[positions] ; GQA repeat ; per-segment causal attention
  (segments from cu_seqlens) ; out @ wo.T

Distribution (8 NeuronCores, tensor-parallel over heads):
  core c owns q-heads [4c,4c+4) and kv-head c (GQA groups align),
  wo is row-sharded; each core emits a partial [2048,4096] bf16 output and
  the host sums the 8 partials in f32.

On-device layout is "transposed" ([feature, seq]) throughout so the
contraction dim always sits on SBUF partitions. The kernel is a software
pipeline over 512-column m-chunks, structured to keep the PE dense (no
>3.4us idle windows, so the HAM clock gate stays at 8/8):

  Phase 1: per chunk, weight-stationary qkv projection (PSUM accumulators
  over 32 contraction tiles) runs dense on the PE while the PREVIOUS
  chunk's RMS stats + RoPE (DVE chains, host-gathered cos/sin, PE
  swap-half permutation) and V transposes execute on the other engines.

  Phase 2: per chunk, scoresT = kT.T @ q per key tile, unnormalized exp
  (scores are O(1)), compile-time segment mask plan, PV accumulated in
  PSUM; the softmax denominator is a gpsimd partition_all_reduce (no PSUM
  banks, no PE); normalization on DVE. The previous chunk's output
  projection is issued after each chunk's attention so its dense matmuls
  fill the attention chain gaps. Chunk 0's attention overlaps the last
  rope block via early release of the projection PSUM pool.

All matmul operands are bf16 or f32r (full PE rate). The segment/causal
structure from cu_seqlens and the rope gather by positions are resolved on
the host at build time; the NEFF is specialized to them.
"""

import os
import sys

import numpy as np

for _p in ("/opt/trn_rl_repo",):
    if os.path.isdir(_p) and _p not in sys.path:
        sys.path.insert(0, _p)

S = 2048
D = 4096
HD = 128
HALF = 64
N_HEADS = 32
N_KV = 8
NCORES = 8
QH = N_HEADS // NCORES          # 4 q heads per core
NO = QH + 2                     # o-tiles per core in qkv projection: q0..q3, k, v
DT = D // 128                   # 32 contraction tiles
MC = S // 512                   # 4 m-chunks of 512
NT = S // 128                   # 16 key tiles
EPS = 1e-6
SCALE = HD ** -0.5

LAST_RESULT = None  # BassKernelResults of the most recent run (for test harness)


def _attention_plan(cu_seqlens):
    """Compile-time mask plan from cu_seqlens.

    Returns (plan, mask_pack):
      plan[mc] = list of (nt, w0, w1, mask_ops); w0/w1 are column offsets
      (multiples of 128, relative to the 512-wide m-chunk) of the contiguous
      valid window; mask_ops = [(j, kind, idx)] for 128-col subtiles needing
      a multiplicative 0/1 mask: kind 'tri' uses the shared causal triangle,
      kind 'host' uses mask_pack[:, idx*128:(idx+1)*128].
    """
    idx = np.arange(S)
    seg = np.searchsorted(np.asarray(cu_seqlens), idx, side="right") - 1
    mask_qk = (seg[:, None] == seg[None, :]) & (idx[:, None] >= idx[None, :])
    mask_t = mask_qk.T  # [n, m]

    plan = []
    tiles = []
    tile_ids = {}
    for mc in range(MC):
        entries = []
        for nt in range(NT):
            blk = mask_t[nt * 128:(nt + 1) * 128, mc * 512:(mc + 1) * 512]
            if not blk.any():
                continue
            js = [j for j in range(4) if blk[:, j * 128:(j + 1) * 128].any()]
            jlo, jhi = min(js), max(js)
            assert js == list(range(jlo, jhi + 1)), "valid window not contiguous"
            mops = []
            for j in range(jlo, jhi + 1):
                sub = blk[:, j * 128:(j + 1) * 128]
                if sub.all():
                    continue
                m0g = mc * 512 + j * 128
                n0g = nt * 128
                if m0g == n0g and np.array_equal(
                    sub, idx[:128][None, :] >= idx[:128][:, None]
                ):
                    mops.append((j, "tri", -1))
                else:
                    key = sub.tobytes()
                    if key not in tile_ids:
                        tile_ids[key] = len(tiles)
                        tiles.append(sub.astype(np.float32))
                    mops.append((j, "host", tile_ids[key]))
            entries.append((nt, jlo * 128, (jhi + 1) * 128, mops))
        assert entries, "every query row attends to at least itself"
        plan.append(entries)

    if tiles:
        mask_pack = np.concatenate(tiles, axis=1)
    else:
        mask_pack = np.zeros((128, 128), dtype=np.float32)
    return plan, np.ascontiguousarray(mask_pack)


def _build_graph(plan, n_mask_cols):
    import concourse.bass as bass  # noqa: PLC0415
    import concourse.mybir as mybir  # noqa: PLC0415
    import concourse.tile as tile  # noqa: PLC0415
    from concourse import bacc, bass_isa  # noqa: PLC0415
    from contextlib import ExitStack  # noqa: PLC0415

    f32 = mybir.dt.float32
    f32r = mybir.dt.float32r
    bf16 = mybir.dt.bfloat16
    AF = mybir.ActivationFunctionType

    nc = bacc.Bacc()
    xT_p = nc.declare_dram_parameter("xT", [D, S], bf16, isOutput=False)
    wqkv_p = nc.declare_dram_parameter("w_qkv", [128, NO * DT * 128], bf16, isOutput=False)
    wo_p = nc.declare_dram_parameter("w_o", [128, QH * D], bf16, isOutput=False)
    cs_p = nc.declare_dram_parameter("cs", [128, 4 * S], bf16, isOutput=False)
    consts_p = nc.declare_dram_parameter("consts", [128, 2 * 128], f32r, isOutput=False)
    constsb_p = nc.declare_dram_parameter(
        "consts_bf", [128, 3 * 128 + n_mask_cols], bf16, isOutput=False
    )
    out_p = nc.declare_dram_parameter("out", [S, D], bf16, isOutput=True)

    with tile.TileContext(nc) as tc, ExitStack() as ctx:
        const = ctx.enter_context(tc.tile_pool(name="const", bufs=1))
        persist = ctx.enter_context(tc.tile_pool(name="persist", bufs=1))
        # rope/stats small-matmul banks; outlives phase 1 — phase 2's output
        # projection reuses these two banks for its PSUM accumulators
        psm = ctx.enter_context(tc.tile_pool(name="smpsum", bufs=2, space="PSUM"))

        consts = const.tile([128, 2 * 128], f32r)
        ones_col = consts[:, 0:1]
        ones_row = consts[0:1, 0:128]
        sca_row = consts[0:1, 128:256]  # all = HD**0.5 (divide-by folds attn scale)

        constsb = const.tile([128, 3 * 128 + n_mask_cols], bf16)
        ones_col_bf = constsb[:, 0:1]
        swp_bf = constsb[:, 128:256]    # swap-halves permutation
        ident_bf = constsb[:, 256:384]  # identity (for PE transpose)
        mask_bf = constsb[:, 384:]

        eps_col = const.tile([128, 1], f32)
        nc.gpsimd.memset(eps_col[:], EPS)

        # full-S persistents: rope'd Q heads and K, transposed V (all bf16)
        kT = persist.tile([128, S], bf16)
        v_sb = persist.tile([128, S], bf16)
        wo_sb = persist.tile([128, QH * D], bf16)
        qbf = [persist.tile([128, S], bf16, tag=f"qbf{h}", name=f"qbf{h}") for h in range(QH)]

        # rotating per-(head, m-chunk) attention-output tiles
        pat = ctx.enter_context(tc.tile_pool(name="attn", bufs=2))

        nc.scalar.dma_start(consts[:], consts_p[:])
        nc.scalar.dma_start(constsb[:], constsb_p[:])

        with ExitStack() as s1:
            # w chunks split across gpsimd (even) / scalar (odd), in d order,
            # so the projection d-loop is never starved.
            pw = s1.enter_context(tc.tile_pool(name="wqkv", bufs=1))
            w_sb = pw.tile([128, NO * DT * 128], bf16)
            NCH = 32
            wchunk = NO * DT * 128 // NCH
            for wci in range(NCH):
                eng = nc.gpsimd if wci % 2 == 0 else nc.scalar
                eng.dma_start(
                    w_sb[:, wci * wchunk:(wci + 1) * wchunk],
                    wqkv_p[:, wci * wchunk:(wci + 1) * wchunk],
                )

            px = s1.enter_context(tc.tile_pool(name="xstream", bufs=8))
            pcs = s1.enter_context(tc.tile_pool(name="csstream", bufs=2))
            pqk = s1.enter_context(tc.tile_pool(name="qkvrot", bufs=2))
            prs = s1.enter_context(tc.tile_pool(name="rsq", bufs=3))
            pscr = s1.enter_context(tc.tile_pool(name="scratch", bufs=2))

            qkv_hist = {}
            cst_hist = {}
            rsq_hist = {}

            def stats_block(mc):
                """RMS stats (sum-of-squares matmul + scalar Sqrt) for chunk
                mc. All Sqrt activations are issued in phase 1 so the scalar
                activation table never thrashes against phase 2's Exp."""
                qkv = qkv_hist[mc]
                rsqs = []
                for o in range(QH + 1):
                    sq = pscr.tile([128, 512], f32r, tag="sq", name="sq")
                    nc.vector.tensor_mul(sq[:], qkv[o][:], qkv[o][:])
                    ss = psm.tile([1, 512], f32, tag="sm", name="ss", padded_shape=[128, 512])
                    nc.tensor.matmul(ss[:], ones_col, sq[:], start=True, stop=True)
                    rsq = prs.tile([1, 512], f32r, tag="rsq", name="rsq", bufs=6)
                    nc.scalar.activation(
                        rsq[:], ss[:], AF.Sqrt, bias=eps_col[0:1, :], scale=1.0 / HD,
                    )
                    rsqs.append(rsq)
                rsq_hist[mc] = rsqs

            def rot_block(mc):
                """RoPE rotation + V transposes for chunk mc (no scalar ops)."""
                msl = slice(mc * 512, (mc + 1) * 512)
                qkv = qkv_hist.pop(mc)
                cst = cst_hist.pop(mc)
                rsqs = rsq_hist.pop(mc)
                for o in range(QH + 1):
                    csb = 0 if o < QH else 2
                    row = sca_row if o < QH else ones_row
                    bp = psm.tile([128, 512], f32, tag="sm", name="bp")
                    nc.tensor.matmul(bp[:], swp_bf, qkv[o][:], start=True, stop=True)
                    t1 = pscr.tile([128, 512], f32, tag="t1", name="t1")
                    nc.vector.tensor_mul(t1[:], qkv[o][:], cst[csb][:])
                    t2 = pscr.tile([128, 512], f32, tag="t2", name="t2")
                    nc.vector.tensor_mul(t2[:], bp[:], cst[csb + 1][:])
                    nc.vector.tensor_add(t1[:], t1[:], t2[:])
                    bc = psm.tile([128, 512], f32, tag="sm", name="bc")
                    nc.tensor.matmul(bc[:], row, rsqs[o][:], start=True, stop=True)
                    rrb = pscr.tile([128, 512], f32, tag="rrb", name="rrb")
                    nc.vector.reciprocal_approx_fast(out=rrb[:], in_=bc[:])
                    dst = qbf[o][:, msl] if o < QH else kT[:, msl]
                    nc.vector.tensor_mul(dst, t1[:], rrb[:])

                for k in range(4):
                    nt = mc * 4 + k
                    tp = psm.tile([128, 128], bf16, tag="sm", name="tp")
                    nc.tensor.transpose(tp[:], qkv[QH + 1][:, k * 128:(k + 1) * 128], ident_bf)
                    nc.vector.tensor_copy(v_sb[:, nt * 128:(nt + 1) * 128], tp[:])

            # ---- phase 1: projection d-loops, rope pipelined one chunk behind
            with ExitStack() as s1p:
                pq = s1p.enter_context(tc.tile_pool(name="qkvpsum", bufs=1, space="PSUM"))
                for mc in range(MC):
                    msl = slice(mc * 512, (mc + 1) * 512)
                    cst = {}
                    for ci in range(4):
                        t = pcs.tile([128, 512], bf16, tag=f"cs{ci}", name=f"cs{ci}")
                        nc.scalar.dma_start(
                            t[:], cs_p[:, ci * S + mc * 512: ci * S + (mc + 1) * 512]
                        )
                        cst[ci] = t
                    cst_hist[mc] = cst

                    accs = [
                        pq.tile([128, 512], f32, tag=f"acc{o}", name=f"acc{o}")
                        for o in range(NO)
                    ]
                    for d in range(DT):
                        xt = px.tile([128, 512], bf16, tag="xt")
                        nc.sync.dma_start(xt[:], xT_p[d * 128:(d + 1) * 128, msl])
                        for o in range(NO):
                            woff = (d * NO + o) * 128
                            nc.tensor.matmul(
                                accs[o][:],
                                w_sb[:, woff:woff + 128],
                                xt[:],
                                start=(d == 0),
                                stop=(d == DT - 1),
                            )

                    qkv = []
                    for o in range(NO):
                        t = pqk.tile([128, 512], bf16, tag=f"qk{o}", name=f"qk{o}")
                        nc.vector.tensor_copy(t[:], accs[o][:])
                        qkv.append(t)
                    qkv_hist[mc] = qkv

                    if mc >= 1:
                        stats_block(mc - 1)
                        rot_block(mc - 1)
                    if mc == 2:
                        # wo prefetch after the startup DMA crunch is over
                        woch = QH * D // 4
                        for wci in range(4):
                            nc.scalar.dma_start(
                                wo_sb[:, wci * woch:(wci + 1) * woch],
                                wo_p[:, wci * woch:(wci + 1) * woch],
                            )
                # last chunk's stats now: every scalar Sqrt precedes phase
                # 2's first Exp, so the activation table loads only twice.
                stats_block(MC - 1)
            # pq released here: phase-2 PSUM pools take its banks, so chunk
            # 0's attention can overlap the final rope block below.

            # ---------------- phase 2: attention + output projection ----------------
            with ExitStack() as s2:
                psco = s2.enter_context(tc.tile_pool(name="scpsum", bufs=2, space="PSUM"))
                pov = s2.enter_context(tc.tile_pool(name="ovpsum", bufs=2, space="PSUM"))
                pden = s2.enter_context(tc.tile_pool(name="denpsum", bufs=1, space="PSUM"))
                pbc = s2.enter_context(tc.tile_pool(name="bcpsum", bufs=1, space="PSUM"))
                pex = s2.enter_context(tc.tile_pool(name="exsbuf", bufs=3))
                pnr = s2.enter_context(tc.tile_pool(name="nrsbuf", bufs=2))
                pys = s2.enter_context(tc.tile_pool(name="ysbuf", bufs=2))

                def attn_block(mc):
                    entries = plan[mc]
                    n_ent = len(entries)
                    attnT = []
                    for h in range(QH):
                        ov = pov.tile([128, 512], f32, tag="ov")
                        den = pden.tile([1, 512], f32, tag="den", padded_shape=[128, 512])
                        for i, (nt, w0, w1, mops) in enumerate(entries):
                            nsl = slice(nt * 128, (nt + 1) * 128)
                            sc = psco.tile([128, 512], f32, tag="sc")
                            nc.tensor.matmul(
                                sc[:, w0:w1], kT[:, nsl],
                                qbf[h][:, mc * 512 + w0: mc * 512 + w1],
                                start=True, stop=True,
                            )
                            ex = pex.tile([128, 512], bf16, tag="ex")
                            nc.scalar.activation(ex[:, w0:w1], sc[:, w0:w1], AF.Exp)
                            for (j, kind, tix) in mops:
                                jsl = slice(j * 128, (j + 1) * 128)
                                if kind == "tri":
                                    # zero strictly-below-diagonal (m < n) entries
                                    nc.gpsimd.affine_select(
                                        out=ex[:, jsl], in_=ex[:, jsl],
                                        compare_op=mybir.AluOpType.is_ge,
                                        fill=0.0, base=0,
                                        pattern=[[1, 128]], channel_multiplier=-1,
                                    )
                                else:
                                    nc.vector.tensor_mul(
                                        ex[:, jsl], ex[:, jsl],
                                        mask_bf[:, tix * 128:(tix + 1) * 128],
                                    )
                            first = i == 0
                            last = i == n_ent - 1
                            nc.tensor.matmul(
                                ov[:, w0:w1], v_sb[:, nsl], ex[:, w0:w1],
                                start=first, stop=last, skip_group_check=True,
                            )
                            nc.tensor.matmul(
                                den[0:1, w0:w1], ones_col_bf, ex[:, w0:w1],
                                start=first, stop=last, skip_group_check=True,
                            )
                        den_sb = pnr.tile([1, 512], f32r, tag="den_sb")
                        nc.vector.tensor_copy(den_sb[:], den[:])
                        bc2 = pbc.tile([128, 512], f32, tag="bc2")
                        nc.tensor.matmul(bc2[:], ones_row, den_sb[:], start=True, stop=True)
                        rrb2 = pnr.tile([128, 512], f32, tag="rrb2")
                        nc.vector.reciprocal_approx_fast(out=rrb2[:], in_=bc2[:])
                        at = pat.tile([128, 512], bf16, tag=f"attnT{h}", name=f"attnT{h}")
                        nc.vector.tensor_mul(at[:], ov[:], rrb2[:])
                        attnT.append(at)
                    return attnT

                def outproj(mc, attnT):
                    for j in range(4):
                        mt = mc * 4 + j
                        tsl = slice(mt * 128, (mt + 1) * 128)
                        jsl = slice(j * 128, (j + 1) * 128)
                        ys = pys.tile([128, D], bf16, tag="ys", name="ys")
                        for ec in range(D // 512):
                            # yp reuses the rope small-matmul banks (rope is
                            # done before any output projection starts)
                            yp = psm.tile([128, 512], f32, tag="sm", name="yp")
                            for t in range(QH):
                                nc.tensor.matmul(
                                    yp[:],
                                    attnT[t][:, jsl],
                                    wo_sb[:, t * D + ec * 512: t * D + (ec + 1) * 512],
                                    start=(t == 0),
                                    stop=(t == QH - 1),
                                )
                            esl = slice(ec * 512, (ec + 1) * 512)
                            # DVE-only: a scalar Copy would thrash the ACT
                            # table against the in-flight Exp activations
                            nc.vector.tensor_copy(ys[:, esl], yp[:])
                        if mc == MC - 1:
                            half = D // 2
                            nc.sync.dma_start(out_p[tsl, 0:half], ys[:, 0:half])
                            nc.scalar.dma_start(out_p[tsl, half:D], ys[:, half:D])
                        else:
                            nc.sync.dma_start(out_p[tsl, :], ys[:])

                # last rope first: its Sqrt activations and small matmuls
                # drain before the first Exp, avoiding ACT-table thrash
                rot_block(MC - 1)
                attnT_hist = {0: attn_block(0)}
                for mc in range(1, MC):
                    attnT_hist[mc] = attn_block(mc)
                    outproj(mc - 1, attnT_hist.pop(mc - 1))
                outproj(MC - 1, attnT_hist.pop(MC - 1))

    nc.finalize()
    return nc


def kernel(x, wq, wk, wv, wo, q_norm_w, k_norm_w, rope_cache, positions, cu_seqlens):
    global LAST_RESULT
    from concourse.bass_utils import run_bass_kernel_spmd  # noqa: PLC0415

    x = np.asarray(x, dtype=np.float32)
    wq = np.asarray(wq, dtype=np.float32)
    wk = np.asarray(wk, dtype=np.float32)
    wv = np.asarray(wv, dtype=np.float32)
    wo = np.asarray(wo, dtype=np.float32)
    q_norm_w = np.asarray(q_norm_w, dtype=np.float32)
    k_norm_w = np.asarray(k_norm_w, dtype=np.float32)
    rope_cache = np.asarray(rope_cache, dtype=np.float32)
    positions = np.asarray(positions)
    cu_seqlens = np.asarray(cu_seqlens)

    import ml_dtypes  # noqa: PLC0415

    # ---- host prep (shared) ----
    xT = np.ascontiguousarray(x[0].T.astype(ml_dtypes.bfloat16))  # [D, S]

    pos = positions.reshape(-1)
    cs = rope_cache[pos]               # [S, HALF, 2]
    cosT = cs[:, :, 0].T               # [HALF, S]
    sinT = cs[:, :, 1].T
    cs1 = np.concatenate([cosT, cosT], axis=0)    # [128, S]
    cs2 = np.concatenate([-sinT, sinT], axis=0)

    def fold(w):
        w = w.reshape(HD, 1)
        wsw = np.concatenate([w[HALF:], w[:HALF]], axis=0)
        return cs1 * w, cs2 * wsw

    cs1q, cs2q = fold(q_norm_w)
    cs1k, cs2k = fold(k_norm_w)
    cs_host = np.ascontiguousarray(
        np.concatenate([cs1q, cs2q, cs1k, cs2k], axis=1).astype(ml_dtypes.bfloat16)
    )  # [128, 4S]

    plan, mask_pack = _attention_plan(cu_seqlens)

    consts_bf = np.zeros((128, 3 * 128 + mask_pack.shape[1]), dtype=np.float32)
    consts_bf[:, 0:128] = 1.0
    swp = np.zeros((128, 128), dtype=np.float32)
    swp[np.arange(128), (np.arange(128) + HALF) % 128] = 1.0
    consts_bf[:, 128:256] = swp
    consts_bf[:, 256:384] = np.eye(128, dtype=np.float32)
    consts_bf[:, 384:] = mask_pack
    consts_bf = consts_bf.astype(ml_dtypes.bfloat16)

    consts = np.zeros((128, 2 * 128), dtype=np.float32)
    consts[:, 0:128] = 1.0
    consts[:, 128:256] = 1.0 / SCALE

    # ---- per-core weight shards ----
    in_maps = []
    for c in range(NCORES):
        w_all = np.concatenate(
            [
                wq[c * QH * HD:(c + 1) * QH * HD],   # [512, D]
                wk[c * HD:(c + 1) * HD],             # [128, D]
                wv[c * HD:(c + 1) * HD],             # [128, D]
            ],
            axis=0,
        )  # [NO*128, D]
        w_host = np.ascontiguousarray(
            w_all.reshape(NO, 128, DT, 128).transpose(3, 2, 0, 1)
            .reshape(128, NO * DT * 128).astype(ml_dtypes.bfloat16)
        )
        wo_c = wo[:, c * QH * HD:(c + 1) * QH * HD].T  # [512, D]
        wo_host = np.ascontiguousarray(
            wo_c.reshape(QH, 128, D).transpose(1, 0, 2)
            .reshape(128, QH * D).astype(ml_dtypes.bfloat16)
        )
        in_maps.append(
            {
                "xT": xT,
                "w_qkv": w_host,
                "w_o": wo_host,
                "cs": cs_host,
                "consts": consts,
                "consts_bf": consts_bf,
            }
        )

    nc = _build_graph(plan, mask_pack.shape[1])
    res = run_bass_kernel_spmd(nc, in_maps, list(range(NCORES)))
    LAST_RESULT = res

    out = res.results[0]["out"].astype(np.float32)
    for c in range(1, NCORES):
        out = out + res.results[c]["out"].astype(np.float32)
    return out.reshape(1, S, D)
